# revision 1
# baseline (speedup 1.0000x reference)
"""GroupedQueryAttention (B=1, T=2048, C=2048, H=16, KVH=4, D=128) on 8 trn2 cores.

Sharding: tensor-parallel over heads. Core c owns q-heads {2c, 2c+1} and kv-head
c//2. Wq/Wk/Wv column-sliced, Wo row-sliced on host; each core computes a full
o_proj partial [2048, 2048] (fp16) and the host sums the 8 partials.

Layout/precision strategy (vs the f32r baseline):
  - x is pre-transposed AND pre-tiled on the host into lhsT layout
    (xt[i*128+p, k*128+u] = x[128i+u, 128k+p]) so the 256 on-chip PE
    transposes + copies of the baseline disappear and each t-tile is one
    contiguous 512KB DMA.
  - q/k path (projections, S=q@k^T) stays f32r: the softmax here is very
    sharp (scale=10 on LayerNormed q,k -> logits sigma~110), so 16-bit
    q/k storage would roughly double the final error.
  - everything after the softmax stats is fp16 (P, V, head outs, Wo,
    output partials): full-rate on the PE, 2x DVE, half DMA.
  - softmax: mask-add (scalar_tensor_tensor) + chunk reduce_max on DVE,
    then ONE Exp pass whose accum_out yields the row sums Z (no second
    exp pass); a DVE tensor_scalar_mul applies 1/Z into fp16 P.
    NOTE: tensor_tensor_reduce with accum_out and Exp with scale=-1 both
    hard-fault the device (NRT_EXEC_UNIT_UNRECOVERABLE) despite passing
    CoreSim — do not re-fuse these.
  - ACT runs exactly two functions (Sqrt in phase 1, Exp in phase 2) so
    the activation table is loaded twice instead of ~67 times.
  - phase 2/3 are software-pipelined with a 1-pair skew over the
    (superblock, head) pairs to keep the PE warm through the softmax.
"""

from contextlib import ExitStack

import numpy as np

import concourse.bass as bass
import concourse.bacc as bacc
import concourse.tile as tile
from concourse import mybir
from concourse import bass_utils

P = 128
T = 2048
C = 2048
NT = T // P       # 16 t-tiles
SB = 512          # superblock width
NSB = T // SB     # 4 superblocks
F32 = mybir.dt.float32
F32R = mybir.dt.float32r
F16 = mybir.dt.float16
AF = mybir.ActivationFunctionType
ALU = mybir.AluOpType
AX = mybir.AxisListType
NEG_BIG = -1.0e30
POS_BIG = 3.0e38

N_CORES = 8

# norm pass engine: gpsimd keeps DVE free; fallback "vector" if gpsimd flakes
NORM_ON_GPSIMD = False
# debug staging: 1 = projections only (dump qT/kT/vv), 2 = +attention (dump
# hoT), 3 = full kernel
STAGE = 3


def _build():
    nc = bacc.Bacc("TRN2", target_bir_lowering=False, debug=False,
                   num_devices=N_CORES)
    xt_d = nc.dram_tensor("xt", [T, C], F32, kind="ExternalInput").ap()
    wqkv_d = nc.dram_tensor("wqkv", [P, NT * 512], F32, kind="ExternalInput").ap()
    wo_d = nc.dram_tensor("wo", [P, 2 * C], F16, kind="ExternalInput").ap()
    fk_d = nc.dram_tensor("fk", [P, 1], F32, kind="ExternalInput").ap()
    masks_d = nc.dram_tensor("masks", [P, 5 * SB], F32, kind="ExternalInput").ap()
    id16_d = nc.dram_tensor("id16", [P, P], F16, kind="ExternalInput").ap()
    id32_d = nc.dram_tensor("id32", [P, P], F32, kind="ExternalInput").ap()
    out_d = nc.dram_tensor("out", [T, C], F16, kind="ExternalOutput").ap()

    def mm(a):
        return a.bitcast(F32R)

    with tile.TileContext(nc) as tc, ExitStack() as ctx:
        const = ctx.enter_context(tc.tile_pool(name="const", bufs=1))
        persist = ctx.enter_context(tc.tile_pool(name="persist", bufs=1))

        id16 = const.tile([P, P], F16, tag="id16")
        nc.sync.dma_start(id16[:], id16_d)
        id32 = const.tile([P, P], F32R, tag="id32")
        nc.sync.dma_start(id32[:], id32_d.bitcast(F32R))
        fk = const.tile([P, 1], F32, tag="fk")
        nc.sync.dma_start(fk[:], fk_d)
        masks = const.tile([P, 5 * SB], F32, tag="masks")
        nc.sync.dma_start(masks[:], masks_d)
        wo = const.tile([P, 2 * C], F16, tag="wo")
        nc.sync.dma_start(wo[:], wo_d)

        qT = persist.tile([P, 2 * T], F32R, tag="qT")   # [d, t] per q-head
        kT = persist.tile([P, T], F32R, tag="kT")       # [d, s] (fk folded)
        vv = persist.tile([P, T], F16, tag="vv")        # s-tile j at cols j*128
        hoT = persist.tile([P, 2 * T], F16, tag="hoT")  # [d, t] per head

        # ------------- Phase 1: projections + LN + q/k transposes ---------
        with tc.tile_pool(name="p1w", bufs=1) as p1w, \
             tc.tile_pool(name="xrow", bufs=2) as xrow_p, \
             tc.tile_pool(name="qln", bufs=2) as qln_p, \
             tc.tile_pool(name="st1", bufs=2) as st1, \
             tc.tile_pool(name="psA", bufs=2, space="PSUM") as psA, \
             tc.tile_pool(name="psB", bufs=4, space="PSUM") as psB:
            wq = p1w.tile([P, NT * 512], F32R, tag="wq")
            nc.sync.dma_start(wq[:], wqkv_d.bitcast(F32R))

            for g in range(4):                      # groups of 4 t-tiles
                qkvs = []
                bnb = st1.tile([P, 24], F32, tag="bnb")    # 6 per (tile,col)
                mvb = st1.tile([P, 24], F32, tag="mvb")    # (mean, var) pairs
                for q in range(4):
                    i = 4 * g + q
                    xr = xrow_p.tile([P, C], F32R, tag="xr")
                    nc.sync.dma_start(xr[:],
                                      xt_d[i * P:(i + 1) * P, :].bitcast(F32R))
                    qkv = psB.tile([P, 512], F32, tag="qkv")
                    for k in range(NT):
                        nc.tensor.matmul(qkv[:], xr[:, k * P:(k + 1) * P],
                                         wq[:, k * 512:(k + 1) * 512],
                                         start=(k == 0), stop=(k == NT - 1))
                    qkvs.append(qkv)
                    for j in range(3):
                        nc.vector.bn_stats(bnb[:, 6 * j:6 * j + 6],
                                           qkv[:, j * P:(j + 1) * P])
                        nc.vector.bn_aggr(
                            mvb[:, 2 * (3 * q + j):2 * (3 * q + j) + 2],
                            bnb[:, 6 * j:6 * j + 6])
                    nc.vector.tensor_copy(vv[:, i * P:(i + 1) * P],
                                          qkv[:, 384:512])
                # batched rstd for the group: 1/sqrt(var + eps) [128, 12]
                vpeb = st1.tile([P, 12], F32, tag="vpeb")
                nc.vector.tensor_scalar_add(vpeb[:], mvb[:, 1::2], 1e-5)
                rcpb = st1.tile([P, 12], F32, tag="rcpb")
                nc.vector.reciprocal(rcpb[:], vpeb[:])
                rstdb = st1.tile([P, 12], F32, tag="rstdb")
                nc.scalar.activation(rstdb[:], rcpb[:], AF.Sqrt)
                for q in range(4):
                    i = 4 * g + q
                    qkv = qkvs[q]
                    qln = qln_p.tile([P, 384], F32R, tag="qln")
                    for j in range(3):
                        nc.vector.tensor_scalar(
                            qln[:, j * P:(j + 1) * P],
                            qkv[:, j * P:(j + 1) * P],
                            mvb[:, 2 * (3 * q + j):2 * (3 * q + j) + 1],
                            rstdb[:, 3 * q + j:3 * q + j + 1],
                            ALU.subtract, ALU.mult)
                    for j in range(3):
                        pt = psA.tile([P, P], F32, tag="pt")
                        nc.tensor.transpose(mm(pt[:]),
                                            qln[:, j * P:(j + 1) * P],
                                            id32[:])
                        if j < 2:
                            nc.vector.tensor_copy(
                                qT[:, j * T + i * P:j * T + (i + 1) * P], pt[:])
                        else:
                            nc.vector.tensor_scalar_mul(
                                kT[:, i * P:(i + 1) * P], pt[:], fk[:])

        # ------------- Phase 2+3: attention + o_proj, 1-pair skew ---------
        if STAGE >= 2:
         with tc.tile_pool(name="sS", bufs=2) as s_pool, \
             tc.tile_pool(name="sPe", bufs=2) as pe_pool, \
             tc.tile_pool(name="sP", bufs=2) as p_pool, \
             tc.tile_pool(name="pts", bufs=3) as pt_pool, \
             tc.tile_pool(name="ob", bufs=4) as ob_pool, \
             tc.tile_pool(name="st2", bufs=24) as st2, \
             tc.tile_pool(name="psS", bufs=3, space="PSUM") as psS, \
             tc.tile_pool(name="psPT", bufs=2, space="PSUM") as psPT, \
             tc.tile_pool(name="psO", bufs=1, space="PSUM") as psO, \
             tc.tile_pool(name="psC", bufs=2, space="PSUM") as psC:

            norm_eng = nc.gpsimd if NORM_ON_GPSIMD else nc.vector

            def stage_a(I, h):
                L = (I + 1) * SB
                S = s_pool.tile([P, 4 * T], F32, tag="S")
                Pe = pe_pool.tile([P, 4 * T], F16, tag="Pe")
                Pb = p_pool.tile([P, 4 * T], F16, tag="Pb")
                Z = st2.tile([P, 4], F32, tag="Z")
                Zi = st2.tile([P, 4], F32, tag="Zi")
                for p in range(4):
                    lq = qT[:, h * T + (I * 4 + p) * P:h * T + (I * 4 + p + 1) * P]
                    cmb = st2.tile([P, 4], F32, tag="cmb")
                    for J in range(I + 1):
                        sp = psS.tile([P, SB], F32, tag="sp")
                        nc.tensor.matmul(sp[:], lq,
                                         kT[:, J * SB:(J + 1) * SB],
                                         start=True, stop=True)
                        msl = (masks[:, p * SB:(p + 1) * SB] if J == I
                               else masks[:, 4 * SB:5 * SB])
                        dst = S[:, p * T + J * SB:p * T + (J + 1) * SB]
                        nc.vector.scalar_tensor_tensor(
                            dst, sp[:], 1.0, msl, ALU.mult, ALU.add)
                        nc.vector.reduce_max(cmb[:, J:J + 1], dst, AX.X)
                    mx = st2.tile([P, 1], F32, tag="mx")
                    nc.vector.tensor_reduce(mx[:], cmb[:, 0:I + 1], AX.X,
                                            ALU.max)
                    nm = st2.tile([P, 1], F32, tag="nm")
                    nc.vector.tensor_scalar_mul(nm[:], mx[:], -1.0)
                    nc.scalar.activation(Pe[:, p * T:p * T + L],
                                         S[:, p * T:p * T + L],
                                         AF.Exp, bias=nm[:],
                                         accum_out=Z[:, p:p + 1])
                nc.vector.reciprocal(Zi[:], Z[:])
                for p in range(4):
                    norm_eng.tensor_scalar_mul(
                        Pb[:, p * T:p * T + L], Pe[:, p * T:p * T + L],
                        Zi[:, p:p + 1])
                return (I, h, Pb)

            def stage_b(I, h, Pb):
                nst = 4 * (I + 1)
                oT = psO.tile([P, SB], F32, tag="oT")
                for j in range(nst):
                    ptp = psPT.tile([P, 2 * SB], F16, tag="ptp")
                    for p in range(4):
                        nc.tensor.transpose(
                            ptp[:, p * P:(p + 1) * P],
                            Pb[:, p * T + j * P:p * T + (j + 1) * P],
                            id16[:])
                    pts = pt_pool.tile([P, SB], F16, tag="pts")
                    nc.vector.tensor_copy(pts[:], ptp[:, 0:SB])
                    nc.tensor.matmul(oT[:], vv[:, j * P:(j + 1) * P], pts[:],
                                     start=(j == 0), stop=(j == nst - 1))
                nc.vector.tensor_copy(
                    hoT[:, h * T + I * SB:h * T + (I + 1) * SB], oT[:])

            def o_proj(I):
                if STAGE < 3:
                    return
                for it in range(4 * I, 4 * I + 4):
                    for e in range(4):
                        po = psC.tile([P, SB], F32, tag="po")
                        for hh in range(2):
                            nc.tensor.matmul(
                                po[:],
                                hoT[:, hh * T + it * P:hh * T + (it + 1) * P],
                                wo[:, hh * C + e * SB:hh * C + (e + 1) * SB],
                                start=(hh == 0), stop=(hh == 1))
                        ob = ob_pool.tile([P, SB], F16, tag="ob")
                        nc.vector.tensor_copy(ob[:], po[:])
                        nc.sync.dma_start(
                            out_d[it * P:(it + 1) * P, e * SB:(e + 1) * SB],
                            ob[:])

            prev = None
            for I in range(NSB):
                for h in range(2):
                    cur = stage_a(I, h)
                    if prev is not None:
                        stage_b(*prev)
                        if prev[1] == 1:
                            o_proj(prev[0])
                    prev = cur
            stage_b(*prev)
            o_proj(NSB - 1)

    nc.compile()
    return nc


def _host_inputs(x, Wq, Wk, Wv, Wo, gq, gk, temp):
    """Build the 8 per-core input maps (host-side shard + layout prep)."""
    x2 = np.ascontiguousarray(np.asarray(x, dtype=np.float32).reshape(T, C))
    # xt[i*128 + p, k*128 + u] = x[128*i + u, 128*k + p]
    xt = np.ascontiguousarray(
        x2.reshape(NT, P, NT, P).transpose(0, 3, 2, 1).reshape(T, C))
    scale = np.float32(min(np.exp(np.float32(temp)), np.float32(50.0)))
    fk = np.ascontiguousarray(
        (np.asarray(gq, np.float32) * np.asarray(gk, np.float32)
         * scale).reshape(P, 1))
    id16 = np.eye(P, dtype=np.float16)
    id32 = np.eye(P, dtype=np.float32)
    masks = np.zeros((P, 5 * SB), dtype=np.float32)
    r = np.arange(P)[:, None]
    c = np.arange(SB)[None, :]
    for p in range(4):
        masks[:, p * SB:(p + 1) * SB] = np.where(c <= P * p + r, 0.0, NEG_BIG)
    in_maps = []
    for core in range(N_CORES):
        q0 = core * 256
        kv0 = (core // 2) * P
        wqkv = np.concatenate([Wq[:, q0:q0 + 256],
                               Wk[:, kv0:kv0 + P],
                               Wv[:, kv0:kv0 + P]], axis=1).astype(np.float32)
        # wqkv_t[p, k*512 + j] = wqkv[128k + p, j]
        wqkv_t = np.ascontiguousarray(
            wqkv.reshape(NT, P, 512).transpose(1, 0, 2).reshape(P, NT * 512))
        # wo_t[p, h*2048 + c] = Wo[q0 + 128h + p, c], fp16
        wo_t = np.ascontiguousarray(
            np.asarray(Wo[q0:q0 + 256, :], np.float32)
            .reshape(2, P, C).transpose(1, 0, 2).reshape(P, 2 * C)
            .astype(np.float16))
        in_maps.append({
            "xt": xt,
            "wqkv": wqkv_t,
            "wo": wo_t,
            "fk": fk,
            "masks": masks,
            "id16": id16,
            "id32": id32,
        })
    return in_maps


_NC_CACHE = {}


def _get_nc():
    if "nc" not in _NC_CACHE:
        _NC_CACHE["nc"] = _build()
    return _NC_CACHE["nc"]


def run(inputs, trace=False):
    nc = _get_nc()
    in_maps = _host_inputs(**inputs)
    res = bass_utils.run_bass_kernel_spmd(
        nc, in_maps, core_ids=list(range(N_CORES)), trace=trace)
    acc = res.results[0]["out"].astype(np.float32)
    for corer in res.results[1:]:
        acc = acc + corer["out"].astype(np.float32)
    return acc.reshape(1, T, C), res


def kernel(**inputs):
    out, _ = run(inputs, trace=False)
    return out

